# revision 1
# baseline (speedup 1.0000x reference)
"""VQ codebook encoding (nn_Encoding) Trainium2 Bass kernel.

Math (per batch b):
  Xf = X[b].reshape(D, N).T                      # [N, D], N = H*W
  SL[n,k] = scale[k] * (||x_n||^2 - 2 x_n.c_k + ||c_k||^2)
  A = softmax_k(SL)                              # no max-subtraction needed (|SL| < ~50)
  E[b,k,:] = sum_n A[n,k] * x_n  -  (sum_n A[n,k]) * c_k

Sharding: data-parallel over B: 16 batches -> 2 per NeuronCore x 8 cores.
No collectives needed; outputs are concatenated on the host.

Device pipeline per batch (distance matmuls fp8, softmax math fp32,
aggregation bf16):
  - M1 (PE):  SL^T chunks [128n, 64k] = Xd-tile-stationary fp8 matmuls vs
              (-2*scale*C*2^8)^T, plus a rank-1 bf16 aug matmul adding
              2^8*scale*(c2-1) (the -1 compensates the ones column folded
              into the squared-norm below). PSUM holds 2^8*(SL - scale*x2').
  - x2 (DVE/ACT/Pool): ||x_n||^2 + 1 via Square+accum / scalar_tensor_tensor
              with accum over the [N, 257] host-transposed X (last column =
              ones, reused by M2). Chunks split across 3 engines for balance.
  - W (Pool): W = scale_k * x2'_n (broadcast mult).
  - combine (DVE): psum = psum * 2^-8 + W  (scalar_tensor_tensor in place).
  - exp (ACT): expS = Exp(psum), PSUM source, bf16 out.
  - Z (DVE):  row-sums + reciprocal; A = expS * Zinv (GPSIMD broadcast mult).
  - M2 (PE):  [E1 | asum] [64, 257] += A_chunk^T-stationary @ [X^T | ones]
              moving (bf16), accumulated over all 72 chunks in one PSUM bank.
  - E = E1 - asum * C  (DVE scalar_tensor_tensor), DMA out fp32.
"""

import numpy as np

import concourse.bacc as bacc
import concourse.mybir as mybir
from concourse.bass_utils import run_bass_kernel_spmd
from concourse.tile import TileContext

# Problem constants (hardcoded per harness contract)
B, D, HH, WW = 16, 256, 96, 96
K = 64
N = HH * WW              # 9216
NC = 8                   # cores
NB = B // NC             # batches per core = 2
NCHUNK = N // 128        # 72 chunks of 128 spatial positions
G = 8                    # chunks per softmax group (psum tile = 1 full bank)
NGROUP = NCHUNK // G     # groups per batch

F32 = mybir.dt.float32
BF16 = mybir.dt.bfloat16
FP8 = mybir.dt.float8e4
NP_BF16 = mybir.dt.np(BF16)
NP_FP8 = mybir.dt.np(FP8)

FP8_SCALE = 256.0        # pre-scale on (-2*scale*C) so fp8 values are normal

_STATE = {}

# Tuning knobs
OPTS = {
    "fp8": True,            # fp8 distance matmuls (else bf16 like baseline)
    # x2 engine assignment per chunk: per-batch counts over NCHUNK=72.
    # GPSIMD legally supports only tensor_tensor (stt/ts fail the ISA
    # engine check), so Pool carries W and x2 stays on DVE/ACT.
    "x2_counts": {"dve": 33, "act": 39, "pool": 0},
    "a_engine": "vector",   # engine for A = expS * Zinv (tt form)
    "w_engine": "gpsimd",   # engine for W = scale * x2 (tt form)
    # Ablation knobs (bisection of the HW bottleneck; output wrong if off)
    "do_m1": True,          # distance matmuls (else only the aug matmul)
    "do_m2": True,          # aggregation matmuls
    "do_x2": True,          # squared-norm chunk ops
    "do_softmax": True,     # W/comb/exp/Zred/recip/A chain
    "dma_once": False,      # hoist X loads out of the timing loop (ablation)
    "dma_small": False,     # same DMA structure, 1/8 bytes (ablation)
    "work_bufs": 4,         # work pool depth
    "psl_bufs": 3,          # SL psum pool depth
    "group_aug": True,      # one sc2 aug matmul per group (vs per chunk)
    "interleave": False,    # interleave the two batches' group pipelines
    "nq": 1,                # DMA slices per tensor per batch
    "x2_group": False,      # squared norms via group Square + 3D reduce
    "a_div": False,         # A = expS / Z via tt divide (skips reciprocal)
    "m1_dr": False,         # fp8 DoubleRow: merge M1's two D-half matmuls
}


def _x2_pattern():
    """Evenly interleaved length-NCHUNK engine assignment for the squared-
    norm chunks, with per-batch counts from OPTS["x2_counts"]."""
    counts = dict(OPTS["x2_counts"])
    assert sum(counts.values()) == NCHUNK
    pat = []
    acc = {k: 0.0 for k in counts}
    left = dict(counts)
    for _ in range(NCHUNK):
        for k in counts:
            acc[k] += counts[k] / NCHUNK
        # error-diffusion pick: most accumulated credit, if quota remains
        eng = max((k for k in counts if left[k] > 0), key=lambda k: acc[k])
        pat.append(eng)
        acc[eng] -= 1.0
        left[eng] -= 1
    return pat


def _build_nc(loop_n=None, unroll=1):
    """loop_n: if set, wrap the whole computation in a For_i repeat loop
    (benchmark variant — measures steady-state HW time per iteration).
    unroll: python-level body repetition (TimelineSim steady-state probe)."""
    nc = bacc.Bacc("TRN2", target_bir_lowering=False, debug=False)

    xdt = FP8 if OPTS["fp8"] else BF16
    # DRAM I/O (per-core shard)
    xd = nc.dram_tensor("xd", [NB, 128, 2 * N], xdt, kind="ExternalInput").ap()
    xto = nc.dram_tensor("xto", [NB, 128, NCHUNK * 257], BF16, kind="ExternalInput").ap()
    cm = nc.dram_tensor("cm", [128, 2 * K], xdt, kind="ExternalInput").ap()
    sc2 = nc.dram_tensor("sc2", [1, K], BF16, kind="ExternalInput").ap()
    sc2g = nc.dram_tensor("sc2g", [1, G * K], BF16, kind="ExternalInput").ap()
    ones = nc.dram_tensor("ones", [1, 128], BF16, kind="ExternalInput").ap()
    scalet = nc.dram_tensor("scalet", [128, K], F32, kind="ExternalInput").ap()
    cw = nc.dram_tensor("cw", [K, D], F32, kind="ExternalInput").ap()
    e_out = nc.dram_tensor("e", [NB, K, D], F32, kind="ExternalOutput").ap()

    with TileContext(nc) as tc:
        with (
            tc.tile_pool(name="const", bufs=1) as constp,
            tc.tile_pool(name="xd", bufs=2) as xdp,
            tc.tile_pool(name="xto", bufs=2) as xtop,
            tc.tile_pool(name="work", bufs=OPTS["work_bufs"]) as workp,
            tc.tile_pool(name="sq", bufs=8) as sqp,
            tc.tile_pool(name="out", bufs=2) as outp,
            tc.tile_pool(name="psl", bufs=OPTS["psl_bufs"], space="PSUM") as pslp,
            tc.tile_pool(name="pe", bufs=2, space="PSUM") as pep,
        ):
            cm_sb = constp.tile([128, 2 * K], xdt)
            sc2_sb = constp.tile([1, K], BF16)
            sc2g_sb = constp.tile([1, G * K], BF16)
            ones_sb = constp.tile([1, 128], BF16)
            scale_sb = constp.tile([128, K], F32)
            cw_sb = constp.tile([K, D], F32)
            nc.sync.dma_start(out=cm_sb[:], in_=cm[:])
            nc.sync.dma_start(out=sc2_sb[:], in_=sc2[:])
            nc.sync.dma_start(out=sc2g_sb[:], in_=sc2g[:])
            nc.sync.dma_start(out=ones_sb[:], in_=ones[:])
            nc.sync.dma_start(out=scale_sb[:], in_=scalet[:])
            nc.sync.dma_start(out=cw_sb[:], in_=cw[:])

            pre_x = None
            if OPTS["dma_once"]:
                pre_x = []
                for b in range(NB):
                    xd_sb = constp.tile([128, 2 * N], xdt)
                    xto_sb = constp.tile([128, NCHUNK * 257], BF16)
                    nc.sync.dma_start(out=xd_sb[:], in_=xd[b])
                    nc.sync.dma_start(out=xto_sb[:], in_=xto[b])
                    pre_x.append((xd_sb, xto_sb))

            import contextlib
            hints = (mybir.EngineType.PE, mybir.EngineType.DVE,
                     mybir.EngineType.Activation, mybir.EngineType.Pool,
                     mybir.EngineType.SP)
            loop_ctx = (tc.For_i(0, loop_n, 1, hint_engines=hints) if loop_n
                        else contextlib.nullcontext())
            with loop_ctx:
                for _ in range(unroll):
                    _kernel_body(nc, tc, locals())

    nc.compile()
    return nc


def _kernel_body(nc, tc, env):
    xd, xto, e_out = env["xd"], env["xto"], env["e_out"]
    xdt = env["xdt"]
    xdp, xtop, workp, sqp, outp = (env["xdp"], env["xtop"], env["workp"],
                                   env["sqp"], env["outp"])
    pslp, pep = env["pslp"], env["pep"]
    cm_sb, sc2_sb, sc2g_sb, ones_sb, scale_sb, cw_sb = (
        env["cm_sb"], env["sc2_sb"], env["sc2g_sb"], env["ones_sb"],
        env["scale_sb"], env["cw_sb"])
    AF = mybir.ActivationFunctionType
    OP = mybir.AluOpType
    AX = mybir.AxisListType
    inv_s = (1.0 / FP8_SCALE) if OPTS["fp8"] else 1.0
    x2pat = _x2_pattern()
    pre_x = env.get("pre_x")
    NQ = OPTS["nq"]                # DMA split: overlap load with compute
    NQC = NCHUNK // NQ             # chunks covered per slice

    def batch_head(b):
        if pre_x is not None:
            xd_sb, xto_sb = pre_x[b]
        else:
            xd_sb = xdp.tile([128, 2 * N], xdt, tag="xd")
            xto_sb = xtop.tile([128, NCHUNK * 257], BF16, tag="xto")
            xdv_s = xd_sb[:].rearrange("p (t n) -> p t n", t=2)
            xdv_d = xd[b].rearrange("p (t n) -> p t n", t=2)
            for q in range(NQ):
                n0, n1 = q * NQC * 128, (q + 1) * NQC * 128
                c0, c1 = q * NQC * 257, (q + 1) * NQC * 257
                if OPTS["dma_small"]:
                    sn, sc = NQC * 16, NQC * 32
                    nc.sync.dma_start(out=xdv_s[:, :, n0:n0 + sn],
                                      in_=xdv_d[:, :, 0:sn])
                    nc.sync.dma_start(out=xto_sb[:, c0:c0 + sc],
                                      in_=xto[b][:, 0:sc])
                else:
                    nc.sync.dma_start(out=xdv_s[:, :, n0:n1],
                                      in_=xdv_d[:, :, n0:n1])
                    nc.sync.dma_start(out=xto_sb[:, c0:c1],
                                      in_=xto[b][:, c0:c1])
        psum_e = pep.tile([K, 257], F32, tag="pe", name="psum_e")
        return {"xd": xd_sb, "xto": xto_sb, "pe": psum_e}

    def group_body(st, b, g):
        xd_sb, xto_sb, psum_e = st["xd"], st["xto"], st["pe"]
        psum_sl = pslp.tile([128, G * K], F32, tag="psl")
        x2g = workp.tile([128, G], F32, tag="x2g")
        w_sb = workp.tile([128, G * K], F32, tag="w")
        expS = workp.tile([128, G * K], BF16, tag="expS")
        zg = workp.tile([128, G], F32, tag="zg")
        zinv_b = workp.tile([128, G], BF16, tag="zinvb")
        a_sb = workp.tile([128, G * K], BF16, tag="a")

        if OPTS["group_aug"]:
            # one rank-1 aug matmul seeds scale*(c2-1) across the whole group
            # (emitted FIRST as the start=True write; the chunk matmuls then
            # accumulate, mirroring the proven start->accum->stop pattern)
            nc.tensor.matmul(
                psum_sl[:], lhsT=ones_sb[:], rhs=sc2g_sb[:],
                start=True, stop=not OPTS["do_m1"])
        if OPTS["x2_group"] and OPTS["do_x2"]:
            # squared norms for the whole group: one Square + one 3D reduce
            sqg = sqp.tile([128, G * 257], BF16, tag="sqg")
            xto_g = xto_sb[:, g * G * 257:(g + 1) * G * 257]
            nc.scalar.activation(sqg[:], xto_g, AF.Square)
            nc.vector.tensor_reduce(
                out=x2g[:], in_=sqg[:].rearrange("p (c f) -> p c f", c=G),
                axis=AX.X, op=OP.add,
            )
        for j in range(G):
            c = g * G + j
            xto_c = xto_sb[:, c * 257:(c + 1) * 257]
            # squared norms (+1 from the ones column), fp32 accum
            eng = x2pat[c]
            if OPTS["x2_group"]:
                pass
            elif not OPTS["do_x2"]:
                if j == 0:
                    nc.vector.memset(x2g[:], 1.0)
            elif eng == "act":
                sq_a = sqp.tile([128, 257], BF16, tag="sq_a")
                nc.scalar.activation(
                    sq_a[:], xto_c, AF.Square,
                    accum_out=x2g[:, j:j + 1],
                )
            else:
                # NOTE: tensor_tensor_reduce hangs on this HW stack;
                # scalar_tensor_tensor with accum_out is equivalent:
                # out = (x * 1) * x, accum = sum(out)
                sq_d = sqp.tile([128, 257], BF16, tag="sq_d")
                nc.vector.scalar_tensor_tensor(
                    out=sq_d[:], in0=xto_c, scalar=1.0, in1=xto_c,
                    op0=OP.mult, op1=OP.mult,
                    accum_out=x2g[:, j:j + 1],
                )
            # M1: SL^T chunk [128n, 64k]
            out_sl = psum_sl[:, j * K:(j + 1) * K]
            if OPTS["do_m1"]:
                ga = OPTS["group_aug"]
                if OPTS["m1_dr"] and OPTS["fp8"]:
                    # one DoubleRow matmul contracts both D-halves (256 rows
                    # as 2 fp8 weights/cell): lhsT free=2M, rhs free=2N
                    xdv3 = xd_sb[:].rearrange("p (t n) -> p t n", t=2)
                    cmv3 = cm_sb[:].rearrange("p (t k) -> p t k", t=2)
                    nc.tensor.matmul(
                        out_sl, lhsT=xdv3[:, :, c * 128:(c + 1) * 128],
                        rhs=cmv3, start=not ga, stop=ga,
                        perf_mode=mybir.MatmulPerfMode.DoubleRow)
                else:
                    nc.tensor.matmul(
                        out_sl, lhsT=xd_sb[:, c * 128:(c + 1) * 128],
                        rhs=cm_sb[:, 0:K], start=not ga, stop=False)
                    nc.tensor.matmul(
                        out_sl, lhsT=xd_sb[:, N + c * 128:N + (c + 1) * 128],
                        rhs=cm_sb[:, K:2 * K], start=False, stop=ga)
                if not ga and not (OPTS["m1_dr"] and OPTS["fp8"]):
                    nc.tensor.matmul(
                        out_sl, lhsT=ones_sb[:], rhs=sc2_sb[:],
                        start=False, stop=True)
            elif not OPTS["group_aug"]:
                nc.tensor.matmul(
                    out_sl, lhsT=ones_sb[:], rhs=sc2_sb[:],
                    start=True, stop=True)

        if not OPTS["do_softmax"]:
            nc.vector.tensor_copy(a_sb[:], xto_sb[:, g * G * K:(g * G + G) * K])
            if OPTS["do_m2"]:
                for j in range(G):
                    c = g * G + j
                    nc.tensor.matmul(
                        psum_e[:], lhsT=a_sb[:, j * K:(j + 1) * K],
                        rhs=xto_sb[:, c * 257:(c + 1) * 257],
                        start=(c == 0), stop=(c == NCHUNK - 1),
                    )
            return
        # W = scale_k * x2'_n  (one batched op per group)
        x2b = x2g[:].to_broadcast((128, G, K))
        scale_rep = scale_sb[:].rearrange(
            "p (o k) -> p o k", o=1).to_broadcast((128, G, K))
        w_eng = nc.gpsimd if OPTS["w_engine"] == "gpsimd" else nc.vector
        a_eng = nc.gpsimd if OPTS["a_engine"] == "gpsimd" else nc.vector
        wv = w_sb[:].rearrange("p (g k) -> p g k", g=G)
        w_eng.tensor_tensor(out=wv, in0=x2b, in1=scale_rep, op=OP.mult)
        # psum = psum * 2^-8 + W  (in place, PSUM src+dst)
        nc.vector.scalar_tensor_tensor(
            out=psum_sl[:], in0=psum_sl[:], scalar=inv_s, in1=w_sb[:],
            op0=OP.mult, op1=OP.add,
        )
        nc.scalar.activation(expS[:], psum_sl[:], AF.Exp)
        nc.vector.tensor_reduce(
            out=zg[:], in_=expS[:].rearrange("p (g k) -> p g k", g=G),
            axis=AX.X, op=OP.add,
        )
        av = a_sb[:].rearrange("p (g k) -> p g k", g=G)
        esv = expS[:].rearrange("p (g k) -> p g k", g=G)
        if OPTS["a_div"]:
            # A = expS / Z directly (one op, skips the reciprocal stage)
            with nc.allow_low_precision(reason="A bf16 divide"):
                a_eng.tensor_tensor(out=av, in0=esv,
                                    in1=zg[:].to_broadcast((128, G, K)),
                                    op=OP.divide)
        else:
            with nc.allow_low_precision(reason="zinv bf16 for A-mult"):
                nc.vector.reciprocal(zinv_b[:], zg[:])
            # A = expS * (1/Z)  (one batched op per group)
            a_eng.tensor_tensor(out=av, in0=esv,
                                in1=zinv_b[:].to_broadcast((128, G, K)),
                                op=OP.mult)
        if OPTS["do_m2"]:
            for j in range(G):
                c = g * G + j
                nc.tensor.matmul(
                    psum_e[:], lhsT=a_sb[:, j * K:(j + 1) * K],
                    rhs=xto_sb[:, c * 257:(c + 1) * 257],
                    start=(c == 0), stop=(c == NCHUNK - 1),
                )

    def batch_tail(st, b):
        psum_e = st["pe"]
        if not OPTS["do_m2"]:
            e_sb = outp.tile([K, D], F32, tag="e_sb")
            nc.vector.tensor_copy(e_sb[:], xto[b] if False else cw_sb[:])
            nc.sync.dma_start(out=e_out[b], in_=e_sb[:])
            return
        # E = E1 - asum * C
        nasum = outp.tile([K, 1], F32, tag="nasum")
        nc.vector.tensor_scalar(
            out=nasum[:], in0=psum_e[:, 256:257],
            scalar1=-1.0, scalar2=None, op0=OP.mult,
        )
        e_sb = outp.tile([K, D], F32, tag="e_sb")
        nc.vector.scalar_tensor_tensor(
            out=e_sb[:], in0=cw_sb[:], scalar=nasum[:],
            in1=psum_e[:, 0:D], op0=OP.mult, op1=OP.add,
        )
        nc.sync.dma_start(out=e_out[b], in_=e_sb[:])

    if OPTS["interleave"]:
        sts = [batch_head(b) for b in range(NB)]
        for g in range(NGROUP):
            for b in range(NB):
                group_body(sts[b], b, g)
        for b in range(NB):
            batch_tail(sts[b], b)
    else:
        for b in range(NB):
            st = batch_head(b)
            for g in range(NGROUP):
                group_body(st, b, g)
            batch_tail(st, b)


def _get_nc(loop_n=None):
    key = ("nc", loop_n)
    if key not in _STATE:
        _STATE[key] = _build_nc(loop_n)
    return _STATE[key]


def _prep_shared(codewords, scale):
    """Host-side constant inputs, keyed by dram tensor name."""
    c2 = (codewords.astype(np.float64) ** 2).sum(1)
    s = FP8_SCALE if OPTS["fp8"] else 1.0
    np_xdt = NP_FP8 if OPTS["fp8"] else NP_BF16
    cm_f = (-2.0 * s * scale[:, None] * codewords).T      # [D, K]
    cm_host = np.ascontiguousarray(
        np.concatenate([cm_f[0:128], cm_f[128:256]], axis=1)
    ).astype(np_xdt)                                       # [128, 2K]
    sc2_host = (s * scale * (c2 - 1.0)).astype(np.float32)[None, :].astype(NP_BF16)
    return {
        "cm": cm_host,
        "sc2": sc2_host,
        "sc2g": np.ascontiguousarray(np.tile(sc2_host, (1, G))),
        "ones": np.ones((1, 128), NP_BF16),
        "scalet": np.ascontiguousarray(
            np.broadcast_to(scale.astype(np.float32)[None, :], (128, K))),
        "cw": np.ascontiguousarray(codewords.astype(np.float32)),
    }


def _prep_core(Xcore):
    """Xcore: [NB, D, H, W] fp32 -> (xd, xto) device layouts."""
    nb = Xcore.shape[0]
    np_xdt = NP_FP8 if OPTS["fp8"] else NP_BF16
    Xf = Xcore.reshape(nb, D, N)
    Xq = Xf.astype(np_xdt)
    # xd: [nb, 128, 2N]; [b, p, t*N + n] = X[b, t*128+p, n]
    xd = np.ascontiguousarray(
        Xq.reshape(nb, 2, 128, N).transpose(0, 2, 1, 3).reshape(nb, 128, 2 * N)
    )
    # xto: [nb, 128, 72*257]; chunk c holds [X^T rows c*128+p | 1.0]
    XT = np.ascontiguousarray(Xf.transpose(0, 2, 1)).astype(NP_BF16)  # [nb, N, D]
    XTO = np.concatenate([XT, np.ones((nb, N, 1), NP_BF16)], axis=2)  # [nb, N, 257]
    xto = np.ascontiguousarray(
        XTO.reshape(nb, NCHUNK, 128, 257).transpose(0, 2, 1, 3).reshape(nb, 128, NCHUNK * 257)
    )
    return xd, xto


def run(X, codewords, scale, trace=False):
    X = np.asarray(X, np.float32)
    codewords = np.asarray(codewords, np.float32)
    scale = np.asarray(scale, np.float32)
    nc = _get_nc()
    shared = _prep_shared(codewords, scale)
    in_maps = []
    for i in range(NC):
        xd_i, xto_i = _prep_core(X[i * NB:(i + 1) * NB])
        in_maps.append({"xd": xd_i, "xto": xto_i, **shared})
    res = run_bass_kernel_spmd(nc, in_maps, list(range(NC)), trace=trace)
    E = np.empty((B, K, D), np.float32)
    for i in range(NC):
        E[i * NB:(i + 1) * NB] = res.results[i]["e"]
    return E, res


def kernel(X, codewords, scale):
    E, _ = run(X, codewords, scale)
    return E

